# revision 4
# baseline (speedup 1.0000x reference)
"""GRU decoder kernel v3 for Trainium2 (Bass/Tile), data-parallel over 8 cores.

Design (see v2 docstring for the folded-H idea; v3 restructures for speed
and precision):
  - Gates r/z/n-pre in folded-H layout [128, 512] fp32 PSUM, produced by
    CONCURRENT bf16 matmul pairs at col groups (0,0)/(0,64).  r/hn/in use
    full-width N=512 slots (2 LDW x 53ns hides under the 213ns stream);
    z is split into two 256-wide half-groups so sigma(z)->u->h' pipelines.
  - Bias rows for step t+1 are pre-issued (start=True) into the PE idle
    window between the step-t transposes.
  - y head runs in f32r natural layout [64, 256] (x/y stay exact fp32;
    W_tp and h' enter y at f32r precision) because the y->x residual chain
    accumulates systematic bf16 weight error coherently over 128 steps.
  - h' transposes: PE pairs, row groups 0/64, separate PSUM banks.
  - psum->SBUF copies: bf16 hT on ACT, f32r hT + xT on GPSIMD (idle).
  - y head of step t is emitted early in iteration t+1 so it overlaps the
    next step's gate matmuls.
"""
import numpy as np

B, T, D, H = 512, 128, 256, 1024
NCORES = 8
BL = B // NCORES  # 64
H3 = 3 * H
KH = H // 128  # 8
KD = D // 128  # 2
HB = 512       # folded column width (H/2)
HQ = 256       # z half width

_CACHE = {}


def _build(nsteps):
    import concourse.bass as bass
    import concourse.mybir as mybir
    import concourse.tile as tile
    from concourse import bacc
    from concourse.masks import make_identity

    F32 = mybir.dt.float32
    F32R = mybir.dt.float32r
    BF16 = mybir.dt.float16  # fp16: 10-bit mantissa, col-tiling legal
    AF = mybir.ActivationFunctionType

    nc = bacc.Bacc(None, target_bir_lowering=False)

    hf0_d = nc.dram_tensor("hf0", [128, HB], F32, kind="ExternalInput")
    hT0_d = nc.dram_tensor("hT0", [128, KH, BL], BF16, kind="ExternalInput")
    hT320_d = nc.dram_tensor("hT320", [128, KH, BL], F32R, kind="ExternalInput")
    x0_d = nc.dram_tensor("x0", [BL, D], F32, kind="ExternalInput")
    xT0_d = nc.dram_tensor("xT0", [128, KD, BL], BF16, kind="ExternalInput")
    whh_d = nc.dram_tensor("whhT", [H, H3], BF16, kind="ExternalInput")
    wih_d = nc.dram_tensor("wihT", [D, H3], BF16, kind="ExternalInput")
    wtp_d = nc.dram_tensor("wtpT", [H, D], F32R, kind="ExternalInput")
    brz_d = nc.dram_tensor("brz", [1, 2 * H], BF16, kind="ExternalInput")
    bni_d = nc.dram_tensor("bni", [1, 2 * H], BF16, kind="ExternalInput")
    btp_d = nc.dram_tensor("btp", [1, D], F32R, kind="ExternalInput")
    ones_d = nc.dram_tensor("ones", [1, 128], BF16, kind="ExternalInput")
    ones32_d = nc.dram_tensor("ones32", [1, 64], F32R, kind="ExternalInput")
    Y_d = nc.dram_tensor("Y", [BL, T, D], F32, kind="ExternalOutput")

    with tile.TileContext(nc) as tc:
        with (
            tc.tile_pool(name="wpool", bufs=1) as wpool,
            tc.tile_pool(name="state", bufs=2) as state,
            tc.tile_pool(name="gates", bufs=1) as gates,
            tc.tile_pool(name="ypool", bufs=3) as ypool,
            tc.tile_pool(name="ps_r", bufs=1, space="PSUM") as ps_r,
            tc.tile_pool(name="ps_za", bufs=1, space="PSUM") as ps_za,
            tc.tile_pool(name="ps_zb", bufs=1, space="PSUM") as ps_zb,
            tc.tile_pool(name="ps_hn", bufs=1, space="PSUM") as ps_hn,
            tc.tile_pool(name="ps_in", bufs=1, space="PSUM") as ps_in,
            tc.tile_pool(name="ps_y", bufs=1, space="PSUM") as ps_y,
            tc.tile_pool(name="ps_ta", bufs=1, space="PSUM") as ps_ta,
            tc.tile_pool(name="ps_tb", bufs=1, space="PSUM") as ps_tb,
        ):
            # --- resident weights
            whh = wpool.tile([128, KH, H3], BF16)
            wih = wpool.tile([128, KD, H3], BF16)
            wtp = wpool.tile([128, KH, D], F32R)
            for c in range(KH):
                nc.sync.dma_start(out=whh[:, c, :], in_=whh_d[c * 128:(c + 1) * 128, :])
                nc.sync.dma_start(out=wtp[:, c, :], in_=wtp_d[c * 128:(c + 1) * 128, :])
            for c in range(KD):
                nc.sync.dma_start(out=wih[:, c, :], in_=wih_d[c * 128:(c + 1) * 128, :])
            brz = wpool.tile([1, 2 * H], BF16)
            bni = wpool.tile([1, 2 * H], BF16)
            btp = wpool.tile([1, D], F32R)
            ones = wpool.tile([1, 128], BF16)
            ones32 = wpool.tile([1, 64], F32R)
            nc.sync.dma_start(out=brz, in_=brz_d[:, :])
            nc.sync.dma_start(out=bni, in_=bni_d[:, :])
            nc.sync.dma_start(out=btp, in_=btp_d[:, :])
            nc.sync.dma_start(out=ones, in_=ones_d[:, :])
            nc.sync.dma_start(out=ones32, in_=ones32_d[:, :])
            ident = wpool.tile([128, 64], F32)
            make_identity(nc, ident[0:64, :])
            make_identity(nc, ident[64:128, :])

            # --- state
            hf = state.tile([128, HB], F32, tag="hf")
            hT = state.tile([128, KH, BL], BF16, tag="hT")
            hT32 = state.tile([128, KH, BL], F32R, tag="hT32")
            xn = state.tile([BL, D], F32, tag="xn")
            xT = state.tile([128, KD, BL], BF16, tag="xT")
            nc.sync.dma_start(out=hf, in_=hf0_d[:, :])
            nc.sync.dma_start(out=hT, in_=hT0_d[:, :, :])
            nc.sync.dma_start(out=hT32, in_=hT320_d[:, :, :])
            nc.sync.dma_start(out=xn, in_=x0_d[:, :])
            nc.sync.dma_start(out=xT, in_=xT0_d[:, :, :])

            HORD = [0, 1, 4, 5, 2, 3, 6, 7]

            def pair(psum, cols, lhsT, rhs0, rhs1, start=False, stop=False):
                nc.tensor.matmul(psum[0:64, cols], lhsT, rhs0, start=start, stop=stop)
                nc.tensor.matmul(psum[64:128, cols], lhsT, rhs1, start=start, stop=stop)

            def bias_pair(psum, cols, bias_ap, b0, b1, width, start, stop):
                nc.tensor.matmul(psum[0:64, cols], ones[:, 0:64],
                                 bias_ap[:, b0:b0 + width], start=start, stop=stop)
                nc.tensor.matmul(psum[64:128, cols], ones[:, 64:128],
                                 bias_ap[:, b1:b1 + width], start=start, stop=stop)

            ALL = slice(0, HB)

            def emit_bias_next():
                """Fresh psum tiles for the next step; start each group with
                its bias row.  Returns the tiles."""
                p_r = ps_r.tile([128, HB], F32, tag="r")
                p_za = ps_za.tile([128, HQ], F32, tag="za")
                p_zb = ps_zb.tile([128, HQ], F32, tag="zb")
                p_hn = ps_hn.tile([128, HB], F32, tag="hn")
                p_in = ps_in.tile([128, HB], F32, tag="in")
                bias_pair(p_r, ALL, brz, 0, HB, HB, True, False)
                bias_pair(p_hn, ALL, bni, 0, HB, HB, True, False)
                bias_pair(p_in, ALL, bni, H, H + HB, HB, True, False)
                for q, p_zq in ((0, p_za), (1, p_zb)):
                    bias_pair(p_zq, slice(0, HQ), brz, H + HQ * q,
                              H + HB + HQ * q, HQ, True, False)
                return p_r, (p_za, p_zb), p_hn, p_in

            def emit_y_head(hT32_t, x_t, t):
                """f32r y head for step t: y = x + h'@W_tp.T + b_tp."""
                p_y = ps_y.tile([BL, D], F32, tag="y")
                for i, c in enumerate(HORD):
                    nc.tensor.matmul(p_y, hT32_t[:, c, :], wtp[:, c, :],
                                     start=(i == 0), stop=False)
                nc.tensor.matmul(p_y, ones32, btp, start=False, stop=True)
                y = ypool.tile([BL, D], F32, tag="y")
                nc.vector.tensor_add(y, x_t, p_y)
                nc.sync.dma_start(out=Y_d[:, t, :], in_=y)
                # xT for the gates of step t+1
                p_ta = ps_ta.tile([128, 5 * BL], F32, tag="ta")
                p_tb = ps_tb.tile([128, 5 * BL], F32, tag="tb")
                nc.tensor.transpose(p_ta[:, 4 * BL:5 * BL], y[:, 0:128],
                                    ident[0:64, :])
                nc.tensor.transpose(p_tb[:, 4 * BL:5 * BL], y[:, 128:256],
                                    ident[0:64, :])
                xT_new = state.tile([128, KD, BL], BF16, tag="xT")
                nc.vector.tensor_copy(xT_new[:, 0, :], p_ta[:, 4 * BL:5 * BL])
                nc.vector.tensor_copy(xT_new[:, 1, :], p_tb[:, 4 * BL:5 * BL])
                return y, xT_new, p_ta, p_tb

            # step 0 bias pre-issue
            p_r, p_z, p_hn, p_in = emit_bias_next()
            pend_y = None  # (hT32_t, x_t, t) for the deferred y head

            for t in range(nsteps):
                # --- r h-chunk slots
                for c in HORD:
                    pair(p_r, ALL, hT[:, c, :], whh[:, c, 0:HB], whh[:, c, HB:H])
                # --- deferred y head of step t-1 (overlaps r matmuls)
                if pend_y is not None:
                    y, xT_new, p_ta, p_tb = emit_y_head(*pend_y)
                    xn, xT = y, xT_new
                # --- r x-chunk slots (need xT of this step) + stop
                pair(p_r, ALL, xT[:, 0, :], wih[:, 0, 0:HB], wih[:, 0, HB:H])
                pair(p_r, ALL, xT[:, 1, :], wih[:, 1, 0:HB], wih[:, 1, HB:H],
                     stop=True)
                # --- hn slots
                for i, c in enumerate(HORD):
                    pair(p_hn, ALL, hT[:, c, :], whh[:, c, 2 * H:2 * H + HB],
                         whh[:, c, 2 * H + HB:H3], stop=(i == KH - 1))
                # --- in slots
                pair(p_in, ALL, xT[:, 0, :], wih[:, 0, 2 * H:2 * H + HB],
                     wih[:, 0, 2 * H + HB:H3])
                pair(p_in, ALL, xT[:, 1, :], wih[:, 1, 2 * H:2 * H + HB],
                     wih[:, 1, 2 * H + HB:H3], stop=True)
                # --- z half-slots (separate banks per half: a bank being
                # matmul-written must not be concurrently read by ACT)
                for q, p_zq in ((0, p_z[0]), (1, p_z[1])):
                    hc = slice(0, HQ)
                    z0 = H + HQ * q
                    z1 = H + HB + HQ * q
                    for c in HORD:
                        pair(p_zq, hc, hT[:, c, :], whh[:, c, z0:z0 + HQ],
                             whh[:, c, z1:z1 + HQ])
                    pair(p_zq, hc, xT[:, 0, :], wih[:, 0, z0:z0 + HQ],
                         wih[:, 0, z1:z1 + HQ])
                    pair(p_zq, hc, xT[:, 1, :], wih[:, 1, z0:z0 + HQ],
                         wih[:, 1, z1:z1 + HQ], stop=True)

                # --- elementwise
                r_sb = gates.tile([128, HB], F32, tag="r")
                z_sb = gates.tile([128, HB], F32, tag="z")
                t1 = gates.tile([128, HB], F32, tag="t1")
                t2 = gates.tile([128, HB], F32, tag="t2")
                n_sb = gates.tile([128, HB], F32, tag="n")
                d_sb = gates.tile([128, HB], F32, tag="d")
                u_sb = gates.tile([128, HB], F32, tag="u")
                hf_new = state.tile([128, HB], F32, tag="hf")
                nc.scalar.activation(r_sb, p_r, AF.Sigmoid)
                nc.vector.tensor_mul(t1, r_sb, p_hn)
                nc.vector.tensor_add(t2, t1, p_in)
                nc.scalar.activation(n_sb, t2, AF.Tanh)
                nc.vector.tensor_sub(d_sb, hf, n_sb)
                for q in (0, 1):
                    hc = slice(HQ * q, HQ * q + HQ)
                    nc.scalar.activation(z_sb[:, hc], p_z[q][:, :], AF.Sigmoid)
                    nc.vector.tensor_mul(u_sb[:, hc], z_sb[:, hc], d_sb[:, hc])
                    nc.vector.tensor_add(hf_new[:, hc], n_sb[:, hc], u_sb[:, hc])

                # --- transposes: pairs (c, c+4), banks ta/tb; first the two
                # chunks of folded half A (cols 0:256), then bias pre-issue
                # for t+1, then half B chunks
                p_ta = ps_ta.tile([128, 5 * BL], F32, tag="ta")
                p_tb = ps_tb.tile([128, 5 * BL], F32, tag="tb")
                hT_new = state.tile([128, KH, BL], BF16, tag="hT")
                hT32_new = state.tile([128, KH, BL], F32R, tag="hT32")
                for c in (0, 1):
                    nc.tensor.transpose(p_ta[:, c * BL:(c + 1) * BL],
                                        hf_new[0:64, c * 128:(c + 1) * 128],
                                        ident[0:64, :])
                    nc.tensor.transpose(p_tb[:, c * BL:(c + 1) * BL],
                                        hf_new[64:128, c * 128:(c + 1) * 128],
                                        ident[64:128, :])
                if t + 1 < nsteps:
                    p_r, p_z, p_hn, p_in = emit_bias_next()
                for c in (2, 3):
                    nc.tensor.transpose(p_ta[:, c * BL:(c + 1) * BL],
                                        hf_new[0:64, c * 128:(c + 1) * 128],
                                        ident[0:64, :])
                    nc.tensor.transpose(p_tb[:, c * BL:(c + 1) * BL],
                                        hf_new[64:128, c * 128:(c + 1) * 128],
                                        ident[64:128, :])
                nc.vector.tensor_copy(hT_new[:, 0:4, :], p_ta[:, 0:4 * BL])
                nc.vector.tensor_copy(hT_new[:, 4:8, :], p_tb[:, 0:4 * BL])
                nc.scalar.copy(hT32_new[:, 0:4, :], p_ta[:, 0:4 * BL])
                nc.scalar.copy(hT32_new[:, 4:8, :], p_tb[:, 0:4 * BL])

                pend_y = (hT32_new, xn, t)
                hf, hT, hT32 = hf_new, hT_new, hT32_new

            # flush the last y head
            emit_y_head(*pend_y)

    nc.finalize()
    return nc


def _build_null():
    import concourse.mybir as mybir
    import concourse.tile as tile
    from concourse import bacc

    F32 = mybir.dt.float32
    F32R = mybir.dt.float32r
    BF16 = mybir.dt.float16  # fp16: 10-bit mantissa, col-tiling legal
    nc = bacc.Bacc(None, target_bir_lowering=False)
    hf0_d = nc.dram_tensor("hf0", [128, HB], F32, kind="ExternalInput")
    nc.dram_tensor("hT0", [128, KH, BL], BF16, kind="ExternalInput")
    nc.dram_tensor("hT320", [128, KH, BL], F32R, kind="ExternalInput")
    nc.dram_tensor("x0", [BL, D], F32, kind="ExternalInput")
    nc.dram_tensor("xT0", [128, KD, BL], BF16, kind="ExternalInput")
    nc.dram_tensor("whhT", [H, H3], BF16, kind="ExternalInput")
    nc.dram_tensor("wihT", [D, H3], BF16, kind="ExternalInput")
    nc.dram_tensor("wtpT", [H, D], F32R, kind="ExternalInput")
    nc.dram_tensor("brz", [1, 2 * H], BF16, kind="ExternalInput")
    nc.dram_tensor("bni", [1, 2 * H], BF16, kind="ExternalInput")
    nc.dram_tensor("btp", [1, D], F32R, kind="ExternalInput")
    nc.dram_tensor("ones", [1, 128], BF16, kind="ExternalInput")
    nc.dram_tensor("ones32", [1, 64], F32R, kind="ExternalInput")
    Y_d = nc.dram_tensor("Y", [BL, T, D], F32, kind="ExternalOutput")
    with tile.TileContext(nc) as tc:
        with tc.tile_pool(name="p", bufs=1) as p:
            tmp = p.tile([128, HB], F32)
            nc.sync.dma_start(out=tmp, in_=hf0_d[:, :])
            nc.sync.dma_start(out=Y_d[:, 0, :], in_=tmp[0:64, 0:256])
    nc.finalize()
    return nc


def _get_nc(nsteps):
    if nsteps not in _CACHE:
        _CACHE[nsteps] = _build(nsteps)
    return _CACHE[nsteps]


def _fold(a):
    w = a.shape[1] // 2
    return np.concatenate([a[:, :w], a[:, w:]], axis=0)


def make_in_maps(h, gt, W_ih, W_hh, b_ih, b_hh, W_tp, b_tp):
    f32 = np.float32
    h = np.asarray(h, f32)
    x0 = np.ascontiguousarray(np.asarray(gt, f32)[:, 0, :])
    whhT = np.ascontiguousarray(np.asarray(W_hh, f32).T).astype(np.float16)
    wihT = np.ascontiguousarray(np.asarray(W_ih, f32).T).astype(np.float16)
    wtpT = np.ascontiguousarray(np.asarray(W_tp, f32).T)
    b_sum = np.asarray(b_ih, f32) + np.asarray(b_hh, f32)
    brz = b_sum[None, :2 * H].astype(np.float16)
    bni = np.concatenate([np.asarray(b_hh, f32)[2 * H:],
                          np.asarray(b_ih, f32)[2 * H:]])[None, :].astype(np.float16)
    btp = np.ascontiguousarray(np.asarray(b_tp, f32)[None, :])
    ones = np.ones((1, 128), np.float16)
    ones32 = np.ones((1, 64), f32)
    in_maps = []
    for c in range(NCORES):
        sl = slice(c * BL, (c + 1) * BL)
        hc = np.ascontiguousarray(h[sl])
        xc = np.ascontiguousarray(x0[sl])
        hTc = np.ascontiguousarray(hc.T.reshape(KH, 128, BL).transpose(1, 0, 2))
        xTc = np.ascontiguousarray(xc.T.reshape(KD, 128, BL).transpose(1, 0, 2))
        in_maps.append({
            "hf0": np.ascontiguousarray(_fold(hc)),
            "hT0": hTc.astype(np.float16),
            "hT320": hTc,
            "x0": xc,
            "xT0": xTc.astype(np.float16),
            "whhT": whhT, "wihT": wihT, "wtpT": wtpT,
            "brz": brz, "bni": bni, "btp": btp,
            "ones": ones, "ones32": ones32,
        })
    return in_maps


def kernel(h, gt, W_ih, W_hh, b_ih, b_hh, W_tp, b_tp, time_steps):
    from concourse.bass_utils import run_bass_kernel_spmd
    nsteps = int(time_steps)
    assert nsteps == T, f"kernel hardcodes T={T}, got {nsteps}"
    nc = _get_nc(nsteps)
    in_maps = make_in_maps(h, gt, W_ih, W_hh, b_ih, b_hh, W_tp, b_tp)
    res = run_bass_kernel_spmd(nc, in_maps, core_ids=list(range(NCORES)),
                               trace=False)
    Y = np.concatenate([res.results[c]["Y"] for c in range(NCORES)], axis=0)
    return Y.astype(np.float32)


# revision 6
# speedup vs baseline: 16.6052x; 16.6052x over previous
"""GRU decoder kernel v3 for Trainium2 (Bass/Tile), data-parallel over 8 cores.

Design (see v2 docstring for the folded-H idea; v3 restructures for speed
and precision):
  - Gates r/z/n-pre in folded-H layout [128, 512] fp32 PSUM, produced by
    CONCURRENT bf16 matmul pairs at col groups (0,0)/(0,64).  r/hn/in use
    full-width N=512 slots (2 LDW x 53ns hides under the 213ns stream);
    z is split into two 256-wide half-groups so sigma(z)->u->h' pipelines.
  - Bias rows for step t+1 are pre-issued (start=True) into the PE idle
    window between the step-t transposes.
  - y head runs in f32r natural layout [64, 256] (x/y stay exact fp32;
    W_tp and h' enter y at f32r precision) because the y->x residual chain
    accumulates systematic bf16 weight error coherently over 128 steps.
  - h' transposes: PE pairs, row groups 0/64, separate PSUM banks.
  - psum->SBUF copies: bf16 hT on ACT, f32r hT + xT on GPSIMD (idle).
  - y head of step t is emitted early in iteration t+1 so it overlaps the
    next step's gate matmuls.
"""
import numpy as np

B, T, D, H = 512, 128, 256, 1024
NCORES = 8
BL = B // NCORES  # 64
H3 = 3 * H
KH = H // 128  # 8
KD = D // 128  # 2
HB = 512       # folded column width (H/2)
HQ = 256       # z half width

_CACHE = {}


def _build(nsteps):
    import concourse.bass as bass
    import concourse.mybir as mybir
    import concourse.tile as tile
    from concourse import bacc
    from concourse.masks import make_identity

    F32 = mybir.dt.float32
    F32R = mybir.dt.float32r
    BF16 = mybir.dt.float16  # fp16: 10-bit mantissa, col-tiling legal
    AF = mybir.ActivationFunctionType

    nc = bacc.Bacc(None, target_bir_lowering=False)

    hf0_d = nc.dram_tensor("hf0", [128, HB], F32, kind="ExternalInput")
    hT0_d = nc.dram_tensor("hT0", [128, KH, BL], BF16, kind="ExternalInput")
    xf0_d = nc.dram_tensor("xf0", [128, D // 2], F32, kind="ExternalInput")
    xT0_d = nc.dram_tensor("xT0", [128, KD, BL], BF16, kind="ExternalInput")
    whh_d = nc.dram_tensor("whhT", [H, H3], BF16, kind="ExternalInput")
    wih_d = nc.dram_tensor("wihT", [D, H3], BF16, kind="ExternalInput")
    wtp_d = nc.dram_tensor("wtpT", [H, D], BF16, kind="ExternalInput")
    brz_d = nc.dram_tensor("brz", [1, 2 * H], BF16, kind="ExternalInput")
    bni_d = nc.dram_tensor("bni", [1, 2 * H], BF16, kind="ExternalInput")
    btp_d = nc.dram_tensor("btp", [1, D], BF16, kind="ExternalInput")
    ones_d = nc.dram_tensor("ones", [1, 128], BF16, kind="ExternalInput")
    Y_d = nc.dram_tensor("Y", [T, 128, D // 2], F32, kind="ExternalOutput")

    with tile.TileContext(nc) as tc:
        with (
            tc.tile_pool(name="wpool", bufs=1) as wpool,
            tc.tile_pool(name="state", bufs=2) as state,
            tc.tile_pool(name="gates", bufs=1) as gates,
            tc.tile_pool(name="ypool", bufs=3) as ypool,
            tc.tile_pool(name="ps_r", bufs=1, space="PSUM") as ps_r,
            tc.tile_pool(name="ps_za", bufs=1, space="PSUM") as ps_za,
            tc.tile_pool(name="ps_zb", bufs=1, space="PSUM") as ps_zb,
            tc.tile_pool(name="ps_hn", bufs=1, space="PSUM") as ps_hn,
            tc.tile_pool(name="ps_in", bufs=1, space="PSUM") as ps_in,
            tc.tile_pool(name="ps_y", bufs=1, space="PSUM") as ps_y,
            tc.tile_pool(name="ps_ta", bufs=1, space="PSUM") as ps_ta,
            tc.tile_pool(name="ps_tb", bufs=1, space="PSUM") as ps_tb,
        ):
            # --- resident weights
            whh = wpool.tile([128, KH, H3], BF16)
            wih = wpool.tile([128, KD, H3], BF16)
            wtp = wpool.tile([128, KH, D], BF16)
            for c in range(KH):
                nc.sync.dma_start(out=whh[:, c, :], in_=whh_d[c * 128:(c + 1) * 128, :])
                nc.sync.dma_start(out=wtp[:, c, :], in_=wtp_d[c * 128:(c + 1) * 128, :])
            for c in range(KD):
                nc.sync.dma_start(out=wih[:, c, :], in_=wih_d[c * 128:(c + 1) * 128, :])
            brz = wpool.tile([1, 2 * H], BF16)
            bni = wpool.tile([1, 2 * H], BF16)
            btp = wpool.tile([1, D], BF16)
            ones = wpool.tile([1, 128], BF16)
            nc.sync.dma_start(out=brz, in_=brz_d[:, :])
            nc.sync.dma_start(out=bni, in_=bni_d[:, :])
            nc.sync.dma_start(out=btp, in_=btp_d[:, :])
            nc.sync.dma_start(out=ones, in_=ones_d[:, :])
            ident = wpool.tile([128, 64], F32)
            make_identity(nc, ident[0:64, :])
            make_identity(nc, ident[64:128, :])

            # --- state
            hf = state.tile([128, HB], F32, tag="hf")
            hT = state.tile([128, KH, BL], BF16, tag="hT")
            xn = state.tile([128, D // 2], F32, tag="xn")
            xT = state.tile([128, KD, BL], BF16, tag="xT")
            nc.sync.dma_start(out=hf, in_=hf0_d[:, :])
            nc.sync.dma_start(out=hT, in_=hT0_d[:, :, :])
            nc.sync.dma_start(out=xn, in_=xf0_d[:, :])
            nc.sync.dma_start(out=xT, in_=xT0_d[:, :, :])

            HORD = [0, 1, 4, 5, 2, 3, 6, 7]

            def pair(psum, cols, lhsT, rhs0, rhs1, start=False, stop=False):
                nc.tensor.matmul(psum[0:64, cols], lhsT, rhs0, start=start, stop=stop)
                nc.tensor.matmul(psum[64:128, cols], lhsT, rhs1, start=start, stop=stop)

            def bias_pair(psum, cols, bias_ap, b0, b1, width, start, stop):
                nc.tensor.matmul(psum[0:64, cols], ones[:, 0:64],
                                 bias_ap[:, b0:b0 + width], start=start, stop=stop)
                nc.tensor.matmul(psum[64:128, cols], ones[:, 64:128],
                                 bias_ap[:, b1:b1 + width], start=start, stop=stop)

            ALL = slice(0, HB)

            def emit_bias_next():
                """Fresh psum tiles for the next step; start each group with
                its bias row.  Returns the tiles."""
                p_r = ps_r.tile([128, HB], F32, tag="r")
                p_za = ps_za.tile([128, HQ], F32, tag="za")
                p_zb = ps_zb.tile([128, HQ], F32, tag="zb")
                p_hn = ps_hn.tile([128, HB], F32, tag="hn")
                p_in = ps_in.tile([128, HB], F32, tag="in")
                return p_r, (p_za, p_zb), p_hn, p_in

            def emit_bias_rhnin(p_r, p_hn, p_in):
                bias_pair(p_r, ALL, brz, 0, HB, HB, True, False)
                bias_pair(p_hn, ALL, bni, 0, HB, HB, True, False)
                bias_pair(p_in, ALL, bni, H, H + HB, HB, True, False)

            def emit_bias_z(p_z):
                for q, p_zq in ((0, p_z[0]), (1, p_z[1])):
                    bias_pair(p_zq, slice(0, HQ), brz, H + HQ * q,
                              H + HB + HQ * q, HQ, True, False)

            def emit_y_head(hT_t, x_t, t):
                """fp16 folded y head for step t: y = x + h'@W_tp.T + b_tp.
                Folded y [128, 128]: parts 0:64 = y cols 0:128 (group0),
                parts 64:128 = y cols 128:256 (group1)."""
                p_y = ps_y.tile([128, D // 2], F32, tag="y")
                for i, c in enumerate(HORD):
                    nc.tensor.matmul(p_y[0:64, :], hT_t[:, c, :],
                                     wtp[:, c, 0:128], start=(i == 0), stop=False)
                    nc.tensor.matmul(p_y[64:128, :], hT_t[:, c, :],
                                     wtp[:, c, 128:256], start=(i == 0), stop=False)
                nc.tensor.matmul(p_y[0:64, :], ones[:, 0:64], btp[:, 0:128],
                                 start=False, stop=True)
                nc.tensor.matmul(p_y[64:128, :], ones[:, 64:128], btp[:, 128:256],
                                 start=False, stop=True)
                y = ypool.tile([128, D // 2], F32, tag="y")
                nc.vector.tensor_add(y, x_t, p_y)
                nc.sync.dma_start(out=Y_d[t, :, :], in_=y)
                # xT for the gates of step t+1 (concurrent row pair)
                p_ta = ps_ta.tile([128, 5 * BL], F32, tag="ta")
                p_tb = ps_tb.tile([128, 5 * BL], F32, tag="tb")
                nc.tensor.transpose(p_ta[:, 4 * BL:5 * BL], y[0:64, :],
                                    ident[0:64, :])
                nc.tensor.transpose(p_tb[:, 4 * BL:5 * BL], y[64:128, :],
                                    ident[64:128, :])
                xT_new = state.tile([128, KD, BL], BF16, tag="xT")
                nc.vector.tensor_copy(xT_new[:, 0, :], p_ta[:, 4 * BL:5 * BL])
                nc.vector.tensor_copy(xT_new[:, 1, :], p_tb[:, 4 * BL:5 * BL])
                return y, xT_new, p_ta, p_tb

            # step 0 bias pre-issue
            p_r, p_z, p_hn, p_in = emit_bias_next()
            emit_bias_rhnin(p_r, p_hn, p_in)
            emit_bias_z(p_z)
            pend_y = None  # (hT_t, x_t, t) for the deferred y head

            for t in range(nsteps):
                # --- r h-chunk slots
                for c in HORD:
                    pair(p_r, ALL, hT[:, c, :], whh[:, c, 0:HB], whh[:, c, HB:H])
                # --- deferred y head of step t-1 (overlaps r matmuls)
                if pend_y is not None:
                    y, xT_new, p_ta, p_tb = emit_y_head(*pend_y)
                    xn, xT = y, xT_new
                # --- r x-chunk slots (need xT of this step) + stop
                pair(p_r, ALL, xT[:, 0, :], wih[:, 0, 0:HB], wih[:, 0, HB:H])
                pair(p_r, ALL, xT[:, 1, :], wih[:, 1, 0:HB], wih[:, 1, HB:H],
                     stop=True)
                # --- hn slots
                for i, c in enumerate(HORD):
                    pair(p_hn, ALL, hT[:, c, :], whh[:, c, 2 * H:2 * H + HB],
                         whh[:, c, 2 * H + HB:H3], stop=(i == KH - 1))
                # --- in slots
                pair(p_in, ALL, xT[:, 0, :], wih[:, 0, 2 * H:2 * H + HB],
                     wih[:, 0, 2 * H + HB:H3])
                pair(p_in, ALL, xT[:, 1, :], wih[:, 1, 2 * H:2 * H + HB],
                     wih[:, 1, 2 * H + HB:H3], stop=True)
                # --- z half-slots (separate banks per half: a bank being
                # matmul-written must not be concurrently read by ACT)
                for q, p_zq in ((0, p_z[0]), (1, p_z[1])):
                    hc = slice(0, HQ)
                    z0 = H + HQ * q
                    z1 = H + HB + HQ * q
                    for c in HORD:
                        pair(p_zq, hc, hT[:, c, :], whh[:, c, z0:z0 + HQ],
                             whh[:, c, z1:z1 + HQ])
                    pair(p_zq, hc, xT[:, 0, :], wih[:, 0, z0:z0 + HQ],
                         wih[:, 0, z1:z1 + HQ])
                    pair(p_zq, hc, xT[:, 1, :], wih[:, 1, z0:z0 + HQ],
                         wih[:, 1, z1:z1 + HQ], stop=True)

                # --- elementwise
                r_sb = gates.tile([128, HB], F32, tag="r")
                z_sb = gates.tile([128, HB], F32, tag="z")
                t1 = gates.tile([128, HB], F32, tag="t1")
                t2 = gates.tile([128, HB], F32, tag="t2")
                n_sb = gates.tile([128, HB], F32, tag="n")
                d_sb = gates.tile([128, HB], F32, tag="d")
                u_sb = gates.tile([128, HB], F32, tag="u")
                hf_new = state.tile([128, HB], F32, tag="hf")
                A = slice(0, HQ)
                Bc = slice(HQ, HB)
                # ACT: sig(rA), sig(rB), tanhA, sig(zA), tanhB, sig(zB)
                # DVE: t1A,t2A,t1B,t2B, dA,uA,h'A, dB,uB,h'B
                nc.scalar.activation(r_sb[:, A], p_r[:, A], AF.Sigmoid)
                nc.scalar.activation(r_sb[:, Bc], p_r[:, Bc], AF.Sigmoid)
                nc.vector.tensor_mul(t1[:, A], r_sb[:, A], p_hn[:, A])
                nc.vector.tensor_add(t2[:, A], t1[:, A], p_in[:, A])
                nc.vector.tensor_mul(t1[:, Bc], r_sb[:, Bc], p_hn[:, Bc])
                nc.vector.tensor_add(t2[:, Bc], t1[:, Bc], p_in[:, Bc])
                nc.scalar.activation(n_sb[:, A], t2[:, A], AF.Tanh)
                nc.scalar.activation(z_sb[:, A], p_z[0][:, :], AF.Sigmoid)
                nc.scalar.activation(n_sb[:, Bc], t2[:, Bc], AF.Tanh)
                nc.scalar.activation(z_sb[:, Bc], p_z[1][:, :], AF.Sigmoid)
                nc.vector.tensor_sub(d_sb[:, A], hf[:, A], n_sb[:, A])
                nc.vector.tensor_mul(u_sb[:, A], z_sb[:, A], d_sb[:, A])
                nc.vector.tensor_add(hf_new[:, A], n_sb[:, A], u_sb[:, A])
                nc.vector.tensor_sub(d_sb[:, Bc], hf[:, Bc], n_sb[:, Bc])
                nc.vector.tensor_mul(u_sb[:, Bc], z_sb[:, Bc], d_sb[:, Bc])
                nc.vector.tensor_add(hf_new[:, Bc], n_sb[:, Bc], u_sb[:, Bc])

                # --- transposes: pairs (c, c+4), banks ta/tb; first the two
                # chunks of folded half A (cols 0:256), then bias pre-issue
                # for t+1, then half B chunks
                p_ta = ps_ta.tile([128, 5 * BL], F32, tag="ta")
                p_tb = ps_tb.tile([128, 5 * BL], F32, tag="tb")
                hT_new = state.tile([128, KH, BL], BF16, tag="hT")
                # fill the PE wait for h'A with next-step bias rows
                if t + 1 < nsteps:
                    p_r, p_z, p_hn, p_in = emit_bias_next()
                    emit_bias_rhnin(p_r, p_hn, p_in)
                if t + 1 < nsteps:
                    emit_bias_z(p_z)
                for c in (0, 1):
                    nc.tensor.transpose(p_ta[:, c * BL:(c + 1) * BL],
                                        hf_new[0:64, c * 128:(c + 1) * 128],
                                        ident[0:64, :])
                    nc.tensor.transpose(p_tb[:, c * BL:(c + 1) * BL],
                                        hf_new[64:128, c * 128:(c + 1) * 128],
                                        ident[64:128, :])
                # A-half copies run inside the h'B window (h'B trails h'A by
                # the DVE chain, so these never overlap trB's bank writes)
                nc.scalar.copy(hT_new[:, 0:2, :], p_ta[:, 0:2 * BL])
                nc.scalar.copy(hT_new[:, 4:6, :], p_tb[:, 0:2 * BL])
                for c in (2, 3):
                    nc.tensor.transpose(p_ta[:, c * BL:(c + 1) * BL],
                                        hf_new[0:64, c * 128:(c + 1) * 128],
                                        ident[0:64, :])
                    nc.tensor.transpose(p_tb[:, c * BL:(c + 1) * BL],
                                        hf_new[64:128, c * 128:(c + 1) * 128],
                                        ident[64:128, :])
                nc.scalar.copy(hT_new[:, 2:4, :], p_ta[:, 2 * BL:4 * BL])
                nc.vector.tensor_copy(hT_new[:, 6:8, :], p_tb[:, 2 * BL:4 * BL])

                pend_y = (hT_new, xn, t)
                hf, hT = hf_new, hT_new

            # flush the last y head
            emit_y_head(*pend_y)

    nc.finalize()
    return nc


def _build_null():
    import concourse.mybir as mybir
    import concourse.tile as tile
    from concourse import bacc

    F32 = mybir.dt.float32
    F32R = mybir.dt.float32r
    BF16 = mybir.dt.float16  # fp16: 10-bit mantissa, col-tiling legal
    nc = bacc.Bacc(None, target_bir_lowering=False)
    hf0_d = nc.dram_tensor("hf0", [128, HB], F32, kind="ExternalInput")
    nc.dram_tensor("hT0", [128, KH, BL], BF16, kind="ExternalInput")
    nc.dram_tensor("xf0", [128, D // 2], F32, kind="ExternalInput")
    nc.dram_tensor("xT0", [128, KD, BL], BF16, kind="ExternalInput")
    nc.dram_tensor("whhT", [H, H3], BF16, kind="ExternalInput")
    nc.dram_tensor("wihT", [D, H3], BF16, kind="ExternalInput")
    nc.dram_tensor("wtpT", [H, D], BF16, kind="ExternalInput")
    nc.dram_tensor("brz", [1, 2 * H], BF16, kind="ExternalInput")
    nc.dram_tensor("bni", [1, 2 * H], BF16, kind="ExternalInput")
    nc.dram_tensor("btp", [1, D], BF16, kind="ExternalInput")
    nc.dram_tensor("ones", [1, 128], BF16, kind="ExternalInput")
    Y_d = nc.dram_tensor("Y", [T, 128, D // 2], F32, kind="ExternalOutput")
    with tile.TileContext(nc) as tc:
        with tc.tile_pool(name="p", bufs=1) as p:
            tmp = p.tile([128, HB], F32)
            nc.sync.dma_start(out=tmp, in_=hf0_d[:, :])
            nc.sync.dma_start(out=Y_d[:, 0, :], in_=tmp[0:64, 0:256])
    nc.finalize()
    return nc


def _get_nc(nsteps):
    if nsteps not in _CACHE:
        _CACHE[nsteps] = _build(nsteps)
    return _CACHE[nsteps]


def _fold(a):
    w = a.shape[1] // 2
    return np.concatenate([a[:, :w], a[:, w:]], axis=0)


def make_in_maps(h, gt, W_ih, W_hh, b_ih, b_hh, W_tp, b_tp):
    f32 = np.float32
    h = np.asarray(h, f32)
    x0 = np.ascontiguousarray(np.asarray(gt, f32)[:, 0, :])
    whhT = np.ascontiguousarray(np.asarray(W_hh, f32).T).astype(np.float16)
    wihT = np.ascontiguousarray(np.asarray(W_ih, f32).T).astype(np.float16)
    wtpT = np.ascontiguousarray(np.asarray(W_tp, f32).T).astype(np.float16)
    b_sum = np.asarray(b_ih, f32) + np.asarray(b_hh, f32)
    brz = b_sum[None, :2 * H].astype(np.float16)
    bni = np.concatenate([np.asarray(b_hh, f32)[2 * H:],
                          np.asarray(b_ih, f32)[2 * H:]])[None, :].astype(np.float16)
    btp = np.ascontiguousarray(np.asarray(b_tp, f32)[None, :]).astype(np.float16)
    ones = np.ones((1, 128), np.float16)
    in_maps = []
    for c in range(NCORES):
        sl = slice(c * BL, (c + 1) * BL)
        hc = np.ascontiguousarray(h[sl])
        xc = np.ascontiguousarray(x0[sl])
        hTc = np.ascontiguousarray(hc.T.reshape(KH, 128, BL).transpose(1, 0, 2))
        xTc = np.ascontiguousarray(xc.T.reshape(KD, 128, BL).transpose(1, 0, 2))
        in_maps.append({
            "hf0": np.ascontiguousarray(_fold(hc)),
            "hT0": hTc.astype(np.float16),
            "xf0": np.ascontiguousarray(_fold(xc)),
            "xT0": xTc.astype(np.float16),
            "whhT": whhT, "wihT": wihT, "wtpT": wtpT,
            "brz": brz, "bni": bni, "btp": btp,
            "ones": ones,
        })
    return in_maps


def kernel(h, gt, W_ih, W_hh, b_ih, b_hh, W_tp, b_tp, time_steps):
    from concourse.bass_utils import run_bass_kernel_spmd
    nsteps = int(time_steps)
    assert nsteps == T, f"kernel hardcodes T={T}, got {nsteps}"
    nc = _get_nc(nsteps)
    in_maps = make_in_maps(h, gt, W_ih, W_hh, b_ih, b_hh, W_tp, b_tp)
    res = run_bass_kernel_spmd(nc, in_maps, core_ids=list(range(NCORES)),
                               trace=False)
    Y = np.concatenate([unfold_Y(res.results[c]["Y"]) for c in range(NCORES)],
                       axis=0)
    return Y.astype(np.float32)


def unfold_Y(Yd):
    """[T, 128, 128] -> [64, T, 256]"""
    out = np.empty((BL, T, D), np.float32)
    out[:, :, :D // 2] = Yd[:, 0:BL, :].transpose(1, 0, 2)
    out[:, :, D // 2:] = Yd[:, BL:128, :].transpose(1, 0, 2)
    return out
